# revision 37
# baseline (speedup 1.0000x reference)
"""Trainium2 Bass kernel for 3x3 VALID conv: x[32,128,64,64] * w[256,128,3,3] + bias.

Strategy (final — 1D Winograd F(2,3) along H):
  - Data-parallel over batch: 8 cores x 4 images; weights/bias replicated.
  - Winograd F(2,3) on the H axis cuts PE matmul columns by 1/3 vs direct:
    for each row-tile it (2 output rows), 4 transformed inputs
        t0 = x[2it]   - x[2it+2]
        t1 = x[2it+1] + x[2it+2]
        t2 = x[2it+2] - x[2it+1]
        t3 = x[2it+1] - x[2it+3]
    feed 4 point-GEMMs (contraction C_IN=128 on partitions, W-taps v=0..2
    accumulated in PSUM):  M_p[o, it, j] = sum_v sum_c gw[p,v][c,o] t_p[c,it,j+v]
    with gw = G @ w along u (host-side, fp32 then bf16).
    Outputs:  y[2it]   = m0 + (m1+bias) + m2
              y[2it+1] = (m1+bias) - m2 - m3
  - Point p=1 is processed FIRST everywhere (weights pre-permuted to
    p1,p0,p2,p3; transform emits t1 first): the first matmul of the kernel
    only waits for one DVE op after the x DMA, and each group's {m1+bias}
    evacuation starts after matmul 3-of-12, overlapping the rest.
  - Engine split per PSUM group (<=8 row-tiles x half of C_OUT):
      PE:  12 matmuls of N=rn*62 cols (4 points x 3 taps)
      ACT: two activations evacuate PSUM->SBUF bf16 — {M1 + per-partition
           bias} (Identity) and {M0,M2,M3} (Copy, one strided AP; PSUM
           point order m1,m0,m2,m3)
      DVE: input transform (4 tensor_tensor) + 4 plain tensor_tensor
           combines (scalar_tensor_tensor has no 2x uop — avoid it)
  - PSUM: 2 groups x [128,4,512] fp32 = 8 banks exactly; groups alternate
    so evacuation overlaps the next group's matmuls.
  - Tail: the last image's groups run h1(5 tiles), h0(7), h1(2) so the
    final group is tiny and flows without PSUM-rotation stalls; it
    pipelines its evacuation between matmuls and stores on Sync (HWDGE).
  - DMA rings: Sync = x chunk0 + w1 + half0 stores; Scalar = w0 (in two
    halves, p1,p0 first) + piece(0,1) then pure evacuation compute;
    GpSimd = bias + remaining pieces + half1 stores.
"""

import numpy as np
import ml_dtypes

import concourse.bacc as bacc
import concourse.tile as tile
from concourse import mybir
from concourse.bass_utils import run_bass_kernel_spmd

N_CORES = 8
B_FULL, C_IN, H, W = 32, 128, 64, 64
C_OUT, KH, KW = 256, 3, 3
B_LOC = B_FULL // N_CORES          # images per core
H_OUT = W_OUT = H - KH + 1         # 62
N_HALF = C_OUT // 128              # 2 output-channel halves
NT = H_OUT // 2                    # 31 row-tiles (2 output rows each)
CHUNKS = [(0, 8), (8, 8), (16, 8), (24, 7)]   # (first tile, tiles in chunk)
N_PTS = 4                          # F(2,3) points
P_ORDER = [1, 0, 2, 3]             # processing order of points
SLOT = {1: 0, 0: 1, 2: 2, 3: 3}    # PSUM/evac/weight slot per point
N_WARM = 7                         # DVFS warm-up matmuls
GW_LEN = N_PTS * KW * 128          # 1536 per half

_cached = {}


def _build_nc():
    f32 = mybir.dt.float32
    bf16 = mybir.dt.bfloat16
    AF = mybir.ActivationFunctionType
    ALU = mybir.AluOpType
    nc = bacc.Bacc()

    w0_d = nc.declare_dram_parameter("w0", [C_IN, GW_LEN], bf16, isOutput=False)
    w1_d = nc.declare_dram_parameter("w1", [C_IN, GW_LEN], bf16, isOutput=False)
    x_d = nc.declare_dram_parameter("x", [B_LOC, C_IN, H, W], bf16, isOutput=False)
    b_d = nc.declare_dram_parameter("bias_in", [128, N_HALF], f32, isOutput=False)
    y_d = nc.declare_dram_parameter(
        "y", [B_LOC, N_HALF, 128, H_OUT, W_OUT], bf16, isOutput=True
    )

    with tile.TileContext(nc) as tc:
        with (
            tc.tile_pool(name="const", bufs=1) as cpool,
            tc.tile_pool(name="xin", bufs=5) as xpool,
            tc.tile_pool(name="tin", bufs=8) as tpool,
            tc.tile_pool(name="mev", bufs=4) as mpool,
            tc.tile_pool(name="yout", bufs=5) as ypool,
            tc.tile_pool(name="tmp", bufs=4) as spool,
            tc.tile_pool(name="psum", bufs=2, space="PSUM") as ppool,
        ):
            w0_t = cpool.tile([C_IN, GW_LEN], bf16)
            w1_t = cpool.tile([C_IN, GW_LEN], bf16)
            x0_t = cpool.tile([C_IN, 18, W], bf16)
            b_t = cpool.tile([128, N_HALF], f32)
            scr = cpool.tile([128, 512], bf16)
            actw = cpool.tile([128, 8], bf16)

            nc.vector.memset(scr[:], 0.0)

            # Startup DMAs, ordered by time-of-need. Sync: img0 chunk0 then
            # half1 taps. Scalar: half0 taps in two slices (p1,p0 first —
            # the matmul order). GpSimd: bias.
            nc.sync.dma_start(x0_t[:], x_d[0, :, 0:18, :])
            nc.sync.dma_start(w1_t[:], w1_d[:])
            nc.scalar.dma_start(w0_t[:, 0 : GW_LEN // 2], w0_d[:, 0 : GW_LEN // 2])
            nc.scalar.dma_start(w0_t[:, GW_LEN // 2 :], w0_d[:, GW_LEN // 2 :])
            nc.gpsimd.dma_start(b_t[:], b_d[:])

            # Preload the ACT function-table set off the critical path.
            nc.scalar.activation(actw[:], scr[:, 0:8], AF.Copy)

            gw0 = w0_t.rearrange("c (t o) -> c t o", t=N_PTS * KW)
            gw1 = w1_t.rearrange("c (t o) -> c t o", t=N_PTS * KW)

            def lhsT(half, p, v):
                gv = gw0 if half == 0 else gw1
                return gv[:, SLOT[p] * KW + v, :]

            # Ramp the PE p-state while the startup DMAs land. Warm-ups
            # write the first psB group buffer (its first real user is the
            # 3rd group, long after these complete). Distinct sizes so no
            # two warm-up matmuls are identical instructions.
            pwarm = ppool.tile([128, 3, 512], f32, tag="psB")
            for i in range(N_WARM):
                nc.tensor.matmul(
                    pwarm[:, i % 3, 0 : 512 - i],
                    scr[:, 0:128],
                    scr[:, 0 : 512 - i],
                    start=True,
                    stop=True,
                )

            piece = {(0, 0): x0_t}

            def load_piece(b, c, eng):
                r0 = 16 * c
                r1 = min(r0 + 18, H)
                px = xpool.tile([C_IN, 18, W], bf16, tag="x")
                eng.dma_start(px[:, 0 : r1 - r0, :], x_d[b, :, r0:r1, :])
                piece[(b, c)] = px

            load_piece(0, 1, nc.scalar)
            load_piece(0, 2, nc.scalar)

            def transform(b, c):
                # DVE input transform for one chunk, point 1 first (matmul
                # order), so the first matmul waits on one op only.
                px = piece[(b, c)]
                ctn = CHUNKS[c][1]
                ts = [
                    tpool.tile([C_IN, 8, W], bf16, tag="t", name=f"t{p}")
                    for p in range(N_PTS)
                ]
                r = lambda a: px[:, a : a + 2 * ctn - 1 : 2, :]
                nc.vector.tensor_tensor(ts[1][:, 0:ctn, :], r(1), r(2), ALU.add)
                nc.vector.tensor_tensor(ts[0][:, 0:ctn, :], r(0), r(2), ALU.subtract)
                nc.vector.tensor_tensor(ts[2][:, 0:ctn, :], r(2), r(1), ALU.subtract)
                nc.vector.tensor_tensor(ts[3][:, 0:ctn, :], r(1), r(3), ALU.subtract)
                return ts

            def do_group(b, half, it0, rn, ts, tr0, store_eng, final=False):
                # One PSUM group: rn row-tiles starting at absolute tile it0,
                # reading ts[p] rows [tr0, tr0+rn). Point 1 accumulates in
                # its own 1-bank tile so its early evacuation doesn't create
                # a false dependency against the other points' matmuls. The
                # final group also gives p3 its own bank so every evacuation
                # except {m3} overlaps matmuls (minimal post-matmul chain).
                n = rn * W_OUT
                psa = ppool.tile([128, 512], f32, tag="psA")
                psb = ppool.tile([128, 3, 512], f32, tag="psB")
                psc = (
                    ppool.tile([128, 512], f32, tag="psA", name="psc")
                    if final
                    else None
                )
                m = mpool.tile([128, N_PTS, 8 * W_OUT], bf16, tag="m")
                yt = ypool.tile([128, 16, W_OUT], bf16, tag="y")
                bh = b_t[:, half : half + 1]
                for p in P_ORDER:
                    if p == 1:
                        dst = psa[:, 0:n]
                    elif final and p == 3:
                        dst = psc[:, 0:n]
                    else:
                        dst = psb[:, SLOT[p] - 1, 0:n]
                    for v in range(KW):
                        nc.tensor.matmul(
                            dst,
                            lhsT(half, p, v),
                            ts[p][:, tr0 : tr0 + rn, v : v + W_OUT],
                            start=(v == 0),
                            stop=(v == KW - 1),
                        )
                    if p == 1:
                        # {m1 + bias} evacuates right after p1's matmuls,
                        # overlapping the remaining 9.
                        nc.scalar.activation(
                            m[:, 0, 0:n], psa[:, 0:n], AF.Identity, bias=bh
                        )
                    elif final and p == 2:
                        nc.scalar.activation(
                            m[:, 1:3, 0:n], psb[:, 0:2, 0:n], AF.Copy
                        )
                if final:
                    nc.scalar.activation(m[:, 3, 0:n], psc[:, 0:n], AF.Copy)
                else:
                    nc.scalar.activation(m[:, 1:4, 0:n], psb[:, :, 0:n], AF.Copy)
                m3d = [
                    m[:, SLOT[p], 0:n].rearrange("c (h w) -> c h w", w=W_OUT)
                    for p in range(N_PTS)
                ]
                y0 = yt[:, 0 : 2 * rn : 2, :]
                y1 = yt[:, 1 : 2 * rn : 2, :]
                s = spool.tile([128, 8, W_OUT], bf16, tag="s")
                sv = s[:, 0:rn, :]
                u = spool.tile([128, 8, W_OUT], bf16, tag="s")
                uv = u[:, 0:rn, :]
                # s and u before y0 so in pipelined mode only y1 waits for
                # the final {m3} evacuation.
                nc.vector.tensor_tensor(sv, m3d[0], m3d[1], ALU.add)
                nc.vector.tensor_tensor(uv, m3d[1], m3d[2], ALU.subtract)
                nc.vector.tensor_tensor(y0, sv, m3d[2], ALU.add)
                nc.vector.tensor_tensor(y1, uv, m3d[3], ALU.subtract)
                store_eng.dma_start(
                    y_d[b, half, :, 2 * it0 : 2 * (it0 + rn), :],
                    yt[:, 0 : 2 * rn, :],
                )

            order = [(b, c) for b in range(B_LOC) for c in range(len(CHUNKS))]
            ts_cur = transform(0, 0)
            for gi, (b, c) in enumerate(order):
                ts_next = transform(*order[gi + 1]) if gi + 1 < len(order) else None
                it0, ctn = CHUNKS[c]
                if gi < len(order) - 1:
                    do_group(b, 0, it0, ctn, ts_cur, 0, nc.sync)
                    do_group(b, 1, it0, ctn, ts_cur, 0, nc.gpsimd)
                else:
                    # Tail: keep the last chunk whole (a 5+2 split adds 12
                    # LDW-bound matmuls to the stream); the final group's
                    # per-point evacuation overlaps its matmuls instead.
                    do_group(b, 0, it0, ctn, ts_cur, 0, nc.sync)
                    do_group(b, 1, it0, ctn, ts_cur, 0, nc.sync, final=True)
                # Piece loads ride the GpSimd queue behind this chunk's h1
                # store so they don't compete with the startup DMAs for
                # SDMA bandwidth (they land ~2 chunks before they're read).
                if gi + 2 < len(order) and order[gi + 2] not in piece:
                    load_piece(*order[gi + 2], nc.gpsimd)
                ts_cur = ts_next

    nc.compile()
    if not nc.is_finalized():
        nc.finalize()
    return nc


_G = np.array(
    [[1.0, 0.0, 0.0], [0.5, 0.5, 0.5], [0.5, -0.5, 0.5], [0.0, 0.0, 1.0]],
    dtype=np.float32,
)


def kernel(inputs, weights, bias, profile=False, trace_kwargs=None):
    x_b = np.ascontiguousarray(
        np.asarray(inputs, dtype=np.float32).astype(ml_dtypes.bfloat16)
    )
    # gw[p,v][c,o]: Winograd-transformed taps, stored in slot order
    # p1,p0,p2,p3 (the processing order): [c, half, slot*3+v, o_local]
    w = np.asarray(weights, dtype=np.float32)
    gw = np.einsum("pu,ocuv->cpvo", _G, w)          # [128, 4, 3, 256]
    gw = gw[:, P_ORDER]                              # slot order p1,p0,p2,p3
    gwh = (
        gw.reshape(C_IN, N_PTS * KW, N_HALF, 128)
        .transpose(0, 2, 1, 3)
        .astype(ml_dtypes.bfloat16)
    )                                                # [128, 2, 12, 128]
    w0_flat = np.ascontiguousarray(gwh[:, 0].reshape(C_IN, GW_LEN))
    w1_flat = np.ascontiguousarray(gwh[:, 1].reshape(C_IN, GW_LEN))
    # [C_OUT, 1] -> [128, N_HALF] with b_t[p, h] = bias[h*128 + p]
    b_t = np.ascontiguousarray(
        np.asarray(bias, dtype=np.float32).reshape(N_HALF, 128).T
    )

    if "nc" not in _cached:
        _cached["nc"] = _build_nc()
    nc = _cached["nc"]

    in_maps = []
    for i in range(N_CORES):
        shard = x_b[i * B_LOC : (i + 1) * B_LOC]
        in_maps.append(
            {"w0": w0_flat, "w1": w1_flat, "x": shard, "bias_in": b_t}
        )
    res = run_bass_kernel_spmd(
        nc,
        in_maps,
        list(range(N_CORES)),
        trace=profile,
        **(trace_kwargs or {}),
    )
    _cached["last_result"] = res

    shards = []
    for i in range(N_CORES):
        y = res.results[i]["y"]  # [B_LOC, 2, 128, 62, 62] bf16
        shards.append(
            np.asarray(y).astype(np.float32).reshape(B_LOC, C_OUT, H_OUT, W_OUT)
        )
    return np.ascontiguousarray(np.concatenate(shards, axis=0), dtype=np.float32)


# revision 38
# speedup vs baseline: 1.1816x; 1.1816x over previous
"""Trainium2 Bass kernel for 3x3 VALID conv: x[32,128,64,64] * w[256,128,3,3] + bias.

Strategy (final — 1D Winograd F(2,3) along H):
  - Data-parallel over batch: 8 cores x 4 images; weights/bias replicated.
  - Winograd F(2,3) on the H axis cuts PE matmul columns by 1/3 vs direct:
    for each row-tile it (2 output rows), 4 transformed inputs
        t0 = x[2it]   - x[2it+2]
        t1 = x[2it+1] + x[2it+2]
        t2 = x[2it+2] - x[2it+1]
        t3 = x[2it+1] - x[2it+3]
    feed 4 point-GEMMs (contraction C_IN=128 on partitions, W-taps v=0..2
    accumulated in PSUM):  M_p[o, it, j] = sum_v sum_c gw[p,v][c,o] t_p[c,it,j+v]
    with gw = G @ w along u (host-side, fp32 then bf16).
    Outputs:  y[2it]   = m0 + (m1+bias) + m2
              y[2it+1] = (m1+bias) - m2 - m3
  - Point p=1 is processed FIRST everywhere (weights pre-permuted to
    p1,p0,p2,p3; transform emits t1 first): the first matmul of the kernel
    only waits for one DVE op after the x DMA, and each group's {m1+bias}
    evacuation starts after matmul 3-of-12, overlapping the rest.
  - Engine split per PSUM group (<=8 row-tiles x half of C_OUT):
      PE:  12 matmuls of N=rn*62 cols (4 points x 3 taps)
      ACT: two activations evacuate PSUM->SBUF bf16 — {M1 + per-partition
           bias} (Identity) and {M0,M2,M3} (Copy, one strided AP; PSUM
           point order m1,m0,m2,m3)
      DVE: input transform (4 tensor_tensor) + 4 plain tensor_tensor
           combines (scalar_tensor_tensor has no 2x uop — avoid it)
  - PSUM: 2 groups x [128,4,512] fp32 = 8 banks exactly; groups alternate
    so evacuation overlaps the next group's matmuls.
  - Tail: the last image's groups run h1(5 tiles), h0(7), h1(2) so the
    final group is tiny and flows without PSUM-rotation stalls; it
    pipelines its evacuation between matmuls and stores on Sync (HWDGE).
  - DMA rings: Sync = x chunk0 + w1 + half0 stores; Scalar = w0 (in two
    halves, p1,p0 first) + piece(0,1) then pure evacuation compute;
    GpSimd = bias + remaining pieces + half1 stores.
"""

import numpy as np
import ml_dtypes

import concourse.bacc as bacc
import concourse.tile as tile
from concourse import mybir
from concourse.bass_utils import run_bass_kernel_spmd

N_CORES = 8
B_FULL, C_IN, H, W = 32, 128, 64, 64
C_OUT, KH, KW = 256, 3, 3
B_LOC = B_FULL // N_CORES          # images per core
H_OUT = W_OUT = H - KH + 1         # 62
N_HALF = C_OUT // 128              # 2 output-channel halves
NT = H_OUT // 2                    # 31 row-tiles (2 output rows each)
CHUNKS = [(0, 8), (8, 8), (16, 8), (24, 7)]   # (first tile, tiles in chunk)
N_PTS = 4                          # F(2,3) points
P_ORDER = [1, 0, 2, 3]             # processing order of points
SLOT = {1: 0, 0: 1, 2: 2, 3: 3}    # PSUM/evac/weight slot per point
N_WARM = 7                         # DVFS warm-up matmuls
GW_LEN = N_PTS * KW * 128          # 1536 per half

_cached = {}


def _build_nc():
    f32 = mybir.dt.float32
    bf16 = mybir.dt.bfloat16
    AF = mybir.ActivationFunctionType
    ALU = mybir.AluOpType
    nc = bacc.Bacc()

    w0_d = nc.declare_dram_parameter("w0", [C_IN, GW_LEN], bf16, isOutput=False)
    w1_d = nc.declare_dram_parameter("w1", [C_IN, GW_LEN], bf16, isOutput=False)
    x_d = nc.declare_dram_parameter("x", [B_LOC, C_IN, H, W], bf16, isOutput=False)
    b_d = nc.declare_dram_parameter("bias_in", [128, N_HALF], f32, isOutput=False)
    y_d = nc.declare_dram_parameter(
        "y", [B_LOC, N_HALF, 128, H_OUT, W_OUT], bf16, isOutput=True
    )

    with tile.TileContext(nc) as tc:
        with (
            tc.tile_pool(name="const", bufs=1) as cpool,
            tc.tile_pool(name="xin", bufs=5) as xpool,
            tc.tile_pool(name="tin", bufs=8) as tpool,
            tc.tile_pool(name="mev", bufs=4) as mpool,
            tc.tile_pool(name="yout", bufs=5) as ypool,
            tc.tile_pool(name="tmp", bufs=4) as spool,
            tc.tile_pool(name="psum", bufs=2, space="PSUM") as ppool,
        ):
            w0_t = cpool.tile([C_IN, GW_LEN], bf16)
            w1_t = cpool.tile([C_IN, GW_LEN], bf16)
            x0_t = cpool.tile([C_IN, 18, W], bf16)
            b_t = cpool.tile([128, N_HALF], f32)
            scr = cpool.tile([128, 512], bf16)
            actw = cpool.tile([128, 8], bf16)

            nc.vector.memset(scr[:], 0.0)

            # Startup DMAs, ordered by time-of-need. Sync: img0 chunk0 then
            # half1 taps. Scalar: half0 taps in two slices (p1,p0 first —
            # the matmul order). GpSimd: bias.
            nc.sync.dma_start(x0_t[:], x_d[0, :, 0:18, :])
            nc.sync.dma_start(w1_t[:], w1_d[:])
            nc.scalar.dma_start(w0_t[:, 0 : GW_LEN // 2], w0_d[:, 0 : GW_LEN // 2])
            nc.scalar.dma_start(w0_t[:, GW_LEN // 2 :], w0_d[:, GW_LEN // 2 :])
            nc.gpsimd.dma_start(b_t[:], b_d[:])

            # Preload the ACT function-table set off the critical path.
            nc.scalar.activation(actw[:], scr[:, 0:8], AF.Copy)

            gw0 = w0_t.rearrange("c (t o) -> c t o", t=N_PTS * KW)
            gw1 = w1_t.rearrange("c (t o) -> c t o", t=N_PTS * KW)

            def lhsT(half, p, v):
                gv = gw0 if half == 0 else gw1
                return gv[:, SLOT[p] * KW + v, :]

            # Ramp the PE p-state while the startup DMAs land. Warm-ups
            # write the first psB group buffer (its first real user is the
            # 3rd group, long after these complete). Distinct sizes so no
            # two warm-up matmuls are identical instructions.
            pwarm = ppool.tile([128, 3, 512], f32, tag="psB")
            for i in range(N_WARM):
                nc.tensor.matmul(
                    pwarm[:, i % 3, 0 : 512 - i],
                    scr[:, 0:128],
                    scr[:, 0 : 512 - i],
                    start=True,
                    stop=True,
                )

            piece = {(0, 0): x0_t}

            def load_piece(b, c, eng):
                r0 = 16 * c
                r1 = min(r0 + 18, H)
                px = xpool.tile([C_IN, 18, W], bf16, tag="x")
                eng.dma_start(px[:, 0 : r1 - r0, :], x_d[b, :, r0:r1, :])
                piece[(b, c)] = px

            load_piece(0, 1, nc.scalar)
            load_piece(0, 2, nc.scalar)

            def transform(b, c):
                # DVE input transform for one chunk, point 1 first (matmul
                # order), so the first matmul waits on one op only.
                px = piece[(b, c)]
                ctn = CHUNKS[c][1]
                ts = [
                    tpool.tile([C_IN, 8, W], bf16, tag="t", name=f"t{p}")
                    for p in range(N_PTS)
                ]
                r = lambda a: px[:, a : a + 2 * ctn - 1 : 2, :]
                nc.vector.tensor_tensor(ts[1][:, 0:ctn, :], r(1), r(2), ALU.add)
                nc.vector.tensor_tensor(ts[0][:, 0:ctn, :], r(0), r(2), ALU.subtract)
                nc.vector.tensor_tensor(ts[2][:, 0:ctn, :], r(2), r(1), ALU.subtract)
                nc.vector.tensor_tensor(ts[3][:, 0:ctn, :], r(1), r(3), ALU.subtract)
                return ts

            def do_group(b, half, it0, rn, ts, tr0, store_eng, final=False):
                # One PSUM group: rn row-tiles starting at absolute tile it0,
                # reading ts[p] rows [tr0, tr0+rn). Point 1 accumulates in
                # its own 1-bank tile so its early evacuation doesn't create
                # a false dependency against the other points' matmuls. The
                # final group also gives p3 its own bank so every evacuation
                # except {m3} overlaps matmuls (minimal post-matmul chain).
                n = rn * W_OUT
                psa = ppool.tile([128, 512], f32, tag="psA")
                psb = ppool.tile([128, 3, 512], f32, tag="psB")
                psc = (
                    ppool.tile([128, 512], f32, tag="psA", name="psc")
                    if final
                    else None
                )
                m = mpool.tile([128, N_PTS, 8 * W_OUT], bf16, tag="m")
                yt = ypool.tile([128, 16, W_OUT], bf16, tag="y")
                bh = b_t[:, half : half + 1]
                for p in P_ORDER:
                    if p == 1:
                        dst = psa[:, 0:n]
                    elif final and p == 3:
                        dst = psc[:, 0:n]
                    else:
                        dst = psb[:, SLOT[p] - 1, 0:n]
                    for v in range(KW):
                        nc.tensor.matmul(
                            dst,
                            lhsT(half, p, v),
                            ts[p][:, tr0 : tr0 + rn, v : v + W_OUT],
                            start=(v == 0),
                            stop=(v == KW - 1),
                        )
                    if p == 1:
                        # {m1 + bias} evacuates right after p1's matmuls,
                        # overlapping the remaining 9.
                        nc.scalar.activation(
                            m[:, 0, 0:n], psa[:, 0:n], AF.Identity, bias=bh
                        )
                    elif final and p == 2:
                        nc.scalar.activation(
                            m[:, 1:3, 0:n], psb[:, 0:2, 0:n], AF.Copy
                        )
                if final:
                    nc.scalar.activation(m[:, 3, 0:n], psc[:, 0:n], AF.Copy)
                else:
                    nc.scalar.activation(m[:, 1:4, 0:n], psb[:, :, 0:n], AF.Copy)
                m3d = [
                    m[:, SLOT[p], 0:n].rearrange("c (h w) -> c h w", w=W_OUT)
                    for p in range(N_PTS)
                ]
                y0 = yt[:, 0 : 2 * rn : 2, :]
                y1 = yt[:, 1 : 2 * rn : 2, :]
                s = spool.tile([128, 8, W_OUT], bf16, tag="s")
                sv = s[:, 0:rn, :]
                u = spool.tile([128, 8, W_OUT], bf16, tag="s")
                uv = u[:, 0:rn, :]
                # s and u before y0 so in pipelined mode only y1 waits for
                # the final {m3} evacuation.
                nc.vector.tensor_tensor(sv, m3d[0], m3d[1], ALU.add)
                nc.vector.tensor_tensor(uv, m3d[1], m3d[2], ALU.subtract)
                nc.vector.tensor_tensor(y0, sv, m3d[2], ALU.add)
                nc.vector.tensor_tensor(y1, uv, m3d[3], ALU.subtract)
                store_eng.dma_start(
                    y_d[b, half, :, 2 * it0 : 2 * (it0 + rn), :],
                    yt[:, 0 : 2 * rn, :],
                )

            order = [(b, c) for b in range(B_LOC) for c in range(len(CHUNKS))]
            ts_cur = transform(0, 0)
            for gi, (b, c) in enumerate(order):
                ts_next = transform(*order[gi + 1]) if gi + 1 < len(order) else None
                it0, ctn = CHUNKS[c]
                if gi < len(order) - 1:
                    do_group(b, 0, it0, ctn, ts_cur, 0, nc.sync)
                    do_group(b, 1, it0, ctn, ts_cur, 0, nc.gpsimd)
                else:
                    # Tail: h1(5), h0(7), then a tiny h1(2) that flows
                    # without PSUM-rotation stalls; last store on Sync.
                    do_group(b, 1, it0, 5, ts_cur, 0, nc.gpsimd)
                    do_group(b, 0, it0, ctn, ts_cur, 0, nc.sync)
                    do_group(b, 1, it0 + 5, 2, ts_cur, 5, nc.sync, final=True)
                # Piece loads ride the GpSimd queue behind this chunk's h1
                # store so they don't compete with the startup DMAs for
                # SDMA bandwidth (they land ~2 chunks before they're read).
                if gi + 2 < len(order) and order[gi + 2] not in piece:
                    load_piece(*order[gi + 2], nc.gpsimd)
                ts_cur = ts_next

    nc.compile()
    if not nc.is_finalized():
        nc.finalize()
    return nc


_G = np.array(
    [[1.0, 0.0, 0.0], [0.5, 0.5, 0.5], [0.5, -0.5, 0.5], [0.0, 0.0, 1.0]],
    dtype=np.float32,
)


def kernel(inputs, weights, bias, profile=False, trace_kwargs=None):
    x_b = np.ascontiguousarray(
        np.asarray(inputs, dtype=np.float32).astype(ml_dtypes.bfloat16)
    )
    # gw[p,v][c,o]: Winograd-transformed taps, stored in slot order
    # p1,p0,p2,p3 (the processing order): [c, half, slot*3+v, o_local]
    w = np.asarray(weights, dtype=np.float32)
    gw = np.einsum("pu,ocuv->cpvo", _G, w)          # [128, 4, 3, 256]
    gw = gw[:, P_ORDER]                              # slot order p1,p0,p2,p3
    gwh = (
        gw.reshape(C_IN, N_PTS * KW, N_HALF, 128)
        .transpose(0, 2, 1, 3)
        .astype(ml_dtypes.bfloat16)
    )                                                # [128, 2, 12, 128]
    w0_flat = np.ascontiguousarray(gwh[:, 0].reshape(C_IN, GW_LEN))
    w1_flat = np.ascontiguousarray(gwh[:, 1].reshape(C_IN, GW_LEN))
    # [C_OUT, 1] -> [128, N_HALF] with b_t[p, h] = bias[h*128 + p]
    b_t = np.ascontiguousarray(
        np.asarray(bias, dtype=np.float32).reshape(N_HALF, 128).T
    )

    if "nc" not in _cached:
        _cached["nc"] = _build_nc()
    nc = _cached["nc"]

    in_maps = []
    for i in range(N_CORES):
        shard = x_b[i * B_LOC : (i + 1) * B_LOC]
        in_maps.append(
            {"w0": w0_flat, "w1": w1_flat, "x": shard, "bias_in": b_t}
        )
    res = run_bass_kernel_spmd(
        nc,
        in_maps,
        list(range(N_CORES)),
        trace=profile,
        **(trace_kwargs or {}),
    )
    _cached["last_result"] = res

    shards = []
    for i in range(N_CORES):
        y = res.results[i]["y"]  # [B_LOC, 2, 128, 62, 62] bf16
        shards.append(
            np.asarray(y).astype(np.float32).reshape(B_LOC, C_OUT, H_OUT, W_OUT)
        )
    return np.ascontiguousarray(np.concatenate(shards, axis=0), dtype=np.float32)
